# revision 42
# baseline (speedup 1.0000x reference)
"""GCNConv mean-aggregation kernel for 8 Trainium2 NeuronCores.

Reference computation:
    msgs   = x[src]                       # [E, D] gather
    summed = segment_sum(msgs, dst, N)    # [N, D]
    deg    = segment_sum(ones, dst, N)    # [N]
    h      = summed / max(deg, 1)
    out    = h @ W.T + b                  # [N, D_OUT]

Strategy (no collectives needed):
  - Shard edges by contiguous dst ranges: core c owns nodes
    [c*6272, (c+1)*6272), 49 windows of 128 dst nodes each.  Each core
    fully reduces its own node range; host reassembles the 8 slices.
  - The gather table holds node PAIRS in bf16: row p = [x[2p], x[2p+1]]
    (256 B, the dma_gather minimum element size), so pair indices fit
    int16 (25001 rows) and a single table serves all 50k nodes.  Each
    128-edge subtile is fetched with one 256B-per-edge dma_gather
    (single_packet=True, 1024 idxs = 64 descs/lane, rotated over the 4
    SWDGE queues) — packet-batched descriptors keep the Pool engine's
    SWDGE descriptor generation (the bottleneck) at its minimum cost.
  - dkey[e] = (dst - window_base) + 128 * (src & 1) in [0, 256) selects
    both the dst slot and which half of the pair the edge uses.  A DVE
    is_equal against a 256-wide iota builds a [128e, 256] bf16 one-hot;
    two PE matmuls per subtile accumulate the transposed window sum
      psT[d, v] += msgs[:, 0:64].T @ oh[:, 0:128]     (even-src edges)
      psT[d, v] += msgs[:, 64:128].T @ oh[:, 128:256] (odd-src edges)
    directly in [64, 128] PSUM — no PE transposes anywhere.
  - Degrees are computed on host (pure index metadata); the kernel
    multiplies psT by a host-provided 1/max(deg,1) row, applies W
    (lhsT = W.T in bf16) and bias, and writes out.T slices.
  - Padding edges use idx 0 (zero row) and dkey -1 (all-zero one-hot).
"""

import sys

sys.path.insert(0, "/opt/trn_rl_repo")

import numpy as np
import ml_dtypes

import concourse.bacc as bacc
import concourse.mybir as mybir
import concourse.tile as tile
from concourse.bass_utils import run_bass_kernel_spmd

N_NODES = 50000
N_EDGES = 800000
D = 64
N_CORES = 8
NPC = 6272          # nodes per core
WIN = 64            # dst-window width per PSUM accumulation group
N_WIN = NPC // WIN  # 98
NPAIR = N_NODES // 2
ROW = 2 * D         # gather row: one node pair in bf16 (256 B)
CHUNK = 8           # subtiles (of 128 edges) per dma_gather call
NQ = 4              # SWDGE queues (ucode max)

F32 = mybir.dt.float32
BF16 = mybir.dt.bfloat16
I16 = mybir.dt.int16
BF = ml_dtypes.bfloat16

# Results of the most recent run (for test harness inspection).
LAST = {}


def _balance_windows(deg_c):
    """Assign the core's NPC nodes to N_WIN windows of exactly WIN nodes,
    best-fit-decreasing on degree so each window's edge count stays at or
    under 8*128 where possible (minimizes 128-edge subtile padding).
    Returns (win_of_node, slot_of_node) local to the core."""
    tight = N_WIN - 4  # windows [tight:] are shared spill (k=9) slots:
    # every core spills into the SAME window indices, so the max-over-cores
    # subtile budget only pays for overflow in 4 windows instead of ~half.
    order = np.argsort(-deg_c, kind="stable")
    total = int(deg_c.sum())
    # Spill nodes: the consecutive run of 4*WIN sorted nodes whose degree
    # sum lands closest to what the tight windows can't absorb at ~1018
    # edges each; the run is mid-degree so spill loads stay under 9*128.
    n_spill = 4 * WIN
    spill_target = total - tight * 1018
    csum = np.zeros(NPC + 1, dtype=np.int64)
    np.cumsum(deg_c[order], out=csum[1:])
    runs = csum[n_spill:] - csum[: NPC - n_spill + 1]
    i0 = int(np.argmin(np.abs(runs - spill_target)))
    is_spill = np.zeros(NPC, dtype=bool)
    is_spill[order[i0 : i0 + n_spill]] = True

    win_of = np.zeros(NPC, dtype=np.int64)
    slot_of = np.zeros(NPC, dtype=np.int64)

    def lpt(nodes, wins):
        loads = np.zeros(len(wins), dtype=np.int64)
        counts = np.zeros(len(wins), dtype=np.int64)
        for n in nodes:  # degree-descending
            open_w = counts < WIN
            cand = np.where(open_w)[0]
            j = cand[np.argmin(loads[cand])]
            win_of[n] = wins[j]
            slot_of[n] = counts[j]
            counts[j] += 1
            loads[j] += deg_c[n]

    lpt(order[~is_spill[order]], np.arange(tight))
    lpt(order[is_spill[order]], np.arange(tight, N_WIN))
    return win_of, slot_of


def _prep(x, src, dst):
    """Host-side sharding: pair gather table, per-core padded edge
    streams (int16 pair idx + f32 dkey), per-window subtile budgets
    (shared across cores; SPMD program structure), 1/deg rows, and the
    node -> output-column permutation from window balancing."""
    x = np.asarray(x, dtype=np.float32)
    src = np.asarray(src, dtype=np.int64)
    dst = np.asarray(dst, dtype=np.int64)

    xp = np.zeros((NPAIR + 1, ROW), dtype=BF)
    xp[1:, :] = x.reshape(NPAIR, ROW).astype(BF)

    deg = np.bincount(dst, minlength=N_CORES * NPC).astype(np.int64)
    inv = (1.0 / np.maximum(deg, 1.0)).astype(np.float32)

    # Balanced node -> (window, slot) assignment per core.
    col_of_node = np.zeros(N_CORES * NPC, dtype=np.int64)
    for c in range(N_CORES):
        win_of, slot_of = _balance_windows(deg[c * NPC : (c + 1) * NPC])
        col_of_node[c * NPC : (c + 1) * NPC] = win_of * WIN + slot_of

    core_of = dst // NPC
    col_d = col_of_node[dst]              # column within the core's out slab
    gw = core_of * N_WIN + col_d // WIN   # global window id
    order = np.argsort(gw, kind="stable")
    src_s = src[order]
    col_s = col_d[order]

    n_groups = N_CORES * N_WIN
    counts = np.bincount(gw[order], minlength=n_groups)
    starts = np.zeros(n_groups + 1, dtype=np.int64)
    np.cumsum(counts, out=starts[1:])

    cnt = counts.reshape(N_CORES, N_WIN)
    k = np.maximum(1, -(-cnt.max(axis=0) // 128))  # [N_WIN]
    SA = int(k.sum())
    offW = np.zeros(N_WIN + 1, dtype=np.int64)
    np.cumsum(k, out=offW[1:])

    idx16 = ((src_s >> 1) + 1).astype(np.int16)
    dkey = ((col_s % WIN) + WIN * (src_s & 1)).astype(np.float32)

    per_core = []
    for c in range(N_CORES):
        iA = np.zeros(SA * 128, dtype=np.int16)
        dK = np.full(SA * 128, -1.0, dtype=np.float32)
        for w in range(N_WIN):
            g = c * N_WIN + w
            s0, s1 = starts[g], starts[g + 1]
            p0 = int(offW[w]) * 128
            iA[p0 : p0 + (s1 - s0)] = idx16[s0:s1]
            dK[p0 : p0 + (s1 - s0)] = dkey[s0:s1]
        inv_c = np.empty(NPC, dtype=np.float32)
        inv_c[col_of_node[c * NPC : (c + 1) * NPC]] = inv[c * NPC : (c + 1) * NPC]
        inv_c = np.tile(inv_c[None, :], (D, 1))
        per_core.append((iA, dK, np.ascontiguousarray(inv_c)))

    return xp, k, SA, offW, per_core, col_of_node


def _wrap_idx(idx_flat):
    """int16 stream -> dma_gather layout [128, n/16]: value i at
    [i % 16, i // 16], replicated across the 8 groups of 16 partitions."""
    a = idx_flat.reshape(-1, 16).T
    return np.tile(a, (8, 1)).copy()


def _wrap_dkey(d_flat):
    """f32 stream -> [128, S]: subtile s lane e at [e, s]."""
    return np.ascontiguousarray(d_flat.reshape(-1, 128).T)


def _build_program(k, SA, offW):
    nc = bacc.Bacc(
        "TRN2", target_bir_lowering=False, debug=False, num_swdge_queues=NQ
    )

    t_xp = nc.dram_tensor("xp", [NPAIR + 1, ROW], BF16, kind="ExternalInput")
    t_wt = nc.dram_tensor("wt", [D, D], BF16, kind="ExternalInput")
    t_b = nc.dram_tensor("bias", [D, 1], F32, kind="ExternalInput")
    t_ia = nc.dram_tensor("idxa", [128, SA * 8], I16, kind="ExternalInput")
    t_dk = nc.dram_tensor("dk", [128, SA], F32, kind="ExternalInput")
    t_iota = nc.dram_tensor("iota", [128, 2 * WIN], F32, kind="ExternalInput")
    t_inv = nc.dram_tensor("invd", [D, NPC], F32, kind="ExternalInput")
    t_out = nc.dram_tensor("out", [D, NPC], F32, kind="ExternalOutput")

    calls = [(p, min(CHUNK, SA - p)) for p in range(0, SA, CHUNK)]

    with tile.TileContext(nc) as tc:
        with (
            tc.tile_pool(name="const", bufs=1) as cpool,
            tc.tile_pool(name="idx", bufs=16) as ipool,
            tc.tile_pool(name="msgs", bufs=6) as mp,
            tc.tile_pool(name="oh", bufs=6) as op,
            tc.tile_pool(name="hout", bufs=3) as hpool,
            tc.tile_pool(name="psacc", bufs=4, space="PSUM") as ps_acc,
            tc.tile_pool(name="psz", bufs=2, space="PSUM") as ps_z,
            tc.tile_pool(name="psiota", bufs=1, space="PSUM") as ps_iota,
        ):
            # Segmented idx/dkey loads first on the sync HWDGE ring: the
            # first gather only waits for its own 128-subtile segment.
            # Constants ride the scalar engine's separate HWDGE ring.
            SEG = 128
            nseg = -(-SA // SEG)
            ia_seg, dk_seg = [], []
            for si in range(nseg):
                s0 = si * SEG
                sl = min(SEG, SA - s0)
                ia_t = ipool.tile([128, sl * 8], I16)
                nc.sync.dma_start(out=ia_t[:], in_=t_ia[:, s0 * 8 : (s0 + sl) * 8])
                ia_seg.append(ia_t)
                dk_t = ipool.tile([128, sl], F32)
                nc.sync.dma_start(out=dk_t[:], in_=t_dk[:, s0 : s0 + sl])
                dk_seg.append(dk_t)

            wt_sb = cpool.tile([D, D], BF16)
            nc.scalar.dma_start(out=wt_sb[:], in_=t_wt[:])
            b_sb = cpool.tile([D, 1], F32)
            nc.scalar.dma_start(out=b_sb[:], in_=t_b[:])
            iota_sb = cpool.tile([128, 2 * WIN], F32)
            nc.scalar.dma_start(out=iota_sb[:], in_=t_iota[:])
            # iota staged in PSUM: the one-hot is_equal then reads PSUM +
            # one SBUF operand, so DVE keeps to its native SBUF port pair
            # and never grabs the shared pair that SWDGE descriptor
            # generation (GpSimd) needs.
            iota_f = ps_iota.tile([128, 2 * WIN], F32)
            nc.vector.tensor_copy(out=iota_f[:], in_=iota_sb[:])
            inv_sb = cpool.tile([D, NPC], F32)
            nc.scalar.dma_start(out=inv_sb[:], in_=t_inv[:])

            # Segmented output slabs so out DMA overlaps the window loop.
            oseg_wins = [25, 25, 24, 24]
            out_tiles = []
            for oi, nw in enumerate(oseg_wins):
                out_tiles.append(
                    cpool.tile([D, nw * WIN], F32, name=f"outsb{oi}")
                )

            chunk_tiles = []
            cursor = [0]

            def emit_chunk(kc):
                pos, nsub = calls[kc]
                si, lo = pos // SEG, pos % SEG
                msgs = mp.tile([128, CHUNK, ROW], BF16)
                nidx = nsub * 128
                # single_packet=True packs 64 descriptors per lane-packet
                # (the spec ceiling; >1024 idxs/call wedges the SDMA
                # engine).  Rotating queue_num spreads ring drain over
                # the 4 SWDGE queues.
                nc.gpsimd.dma_gather(
                    msgs[:, :nsub, :],
                    t_xp[:],
                    ia_seg[si][:, lo * 8 : lo * 8 + nsub * 8],
                    nidx,
                    nidx,
                    ROW,
                    single_packet=True,
                    queue_num=kc % NQ,
                )
                oh = op.tile([128, CHUNK, 2 * WIN], BF16)
                dst_b = (
                    dk_seg[si][:, lo : lo + nsub]
                    .unsqueeze(2)
                    .to_broadcast([128, nsub, 2 * WIN])
                )
                nc.vector.tensor_tensor(
                    out=oh[:, :nsub, :],
                    in0=iota_f[:].unsqueeze(1).to_broadcast([128, nsub, 2 * WIN]),
                    in1=dst_b,
                    op=mybir.AluOpType.is_equal,
                )
                chunk_tiles.append((msgs, oh))

            def tiles_for(s):
                kc = s // CHUNK
                while cursor[0] <= kc:
                    emit_chunk(cursor[0])
                    cursor[0] += 1
                msgs, oh = chunk_tiles[kc]
                return msgs, oh, s % CHUNK

            oseg_base = [0]
            for nw in oseg_wins:
                oseg_base.append(oseg_base[-1] + nw)

            for w in range(N_WIN):
                nsubs = int(k[w])
                ps = ps_acc.tile([D, WIN], F32)
                for j in range(nsubs):
                    msgs, oh, col = tiles_for(int(offW[w]) + j)
                    nc.tensor.matmul(
                        out=ps[:],
                        lhsT=msgs[:, col, 0:D],
                        rhs=oh[:, col, 0:WIN],
                        start=(j == 0),
                        stop=False,
                    )
                    nc.tensor.matmul(
                        out=ps[:],
                        lhsT=msgs[:, col, D:ROW],
                        rhs=oh[:, col, WIN : 2 * WIN],
                        start=False,
                        stop=(j == nsubs - 1),
                    )
                ht = hpool.tile([D, WIN], BF16)
                nc.vector.tensor_tensor(
                    out=ht[:],
                    in0=ps[:],
                    in1=inv_sb[:, w * WIN : (w + 1) * WIN],
                    op=mybir.AluOpType.mult,
                )
                z = ps_z.tile([D, WIN], F32)
                nc.tensor.matmul(
                    out=z[:], lhsT=wt_sb[:], rhs=ht[:], start=True, stop=True
                )
                oi = next(i for i in range(4) if w < oseg_base[i + 1])
                ot = out_tiles[oi]
                nc.vector.tensor_scalar_add(
                    ot[:, (w - oseg_base[oi]) * WIN : (w - oseg_base[oi] + 1) * WIN],
                    z[:],
                    b_sb[:],
                )
                if w == oseg_base[oi + 1] - 1:
                    nc.scalar.dma_start(
                        out=t_out[
                            :, oseg_base[oi] * WIN : oseg_base[oi + 1] * WIN
                        ],
                        in_=ot[:],
                    )

    nc.compile()
    return nc


def kernel(x, src, dst, W, b):
    x = np.asarray(x, dtype=np.float32)
    W = np.asarray(W, dtype=np.float32)
    b = np.asarray(b, dtype=np.float32)

    xp, k, SA, offW, per_core, col_of_node = _prep(x, src, dst)
    nc = _build_program(k, SA, offW)

    wt = np.ascontiguousarray(W.T).astype(BF)
    bcol = np.ascontiguousarray(b.reshape(D, 1))
    iota_arr = np.tile(
        np.arange(2 * WIN, dtype=np.float32)[None, :], (128, 1)
    )

    in_maps = []
    for c in range(N_CORES):
        iA, dK, inv_c = per_core[c]
        in_maps.append(
            {
                "xp": xp,
                "wt": wt,
                "bias": bcol,
                "idxa": _wrap_idx(iA),
                "dk": _wrap_dkey(dK),
                "iota": iota_arr,
                "invd": inv_c,
            }
        )

    res = run_bass_kernel_spmd(nc, in_maps, list(range(N_CORES)))
    LAST["results"] = res
    LAST["exec_time_ns"] = res.exec_time_ns

    out_t = np.concatenate([res.results[c]["out"] for c in range(N_CORES)], axis=1)
    nodes = np.arange(N_NODES)
    cols = (nodes // NPC) * NPC + col_of_node[:N_NODES]
    return np.ascontiguousarray(out_t.T[cols])
